# revision 24
# baseline (speedup 1.0000x reference)
"""BrainAgeGAT Trainium2 kernel: 2-layer GATv2 + mean-pool + MLP on 8 NeuronCores.

Strategy (v2):
  - Edges (incl. self loops) sharded by destination across the 8 cores; within
    a core, destination nodes are LPT-packed into 50 blocks of <=127 slots
    (slot 127 = garbage) so per-block edge counts are balanced and every block
    uses the same uniform tile counts (tba piece-A tiles + tbb piece-B tiles).
  - xl = x@Wl is AllGather'd; per edge a 512-byte bf16 row is fetched with
    dma_gather (SWDGE, int16 indices; the 51200-row table is split in two
    25600-row halves to stay within int16).
  - xr[dst] is NOT gathered: per block the 128-row xr slice is SBUF-resident
    and expanded per edge with a one-hot matmul (lhsT = OtT streamed from
    host) into PSUM, then copied to SBUF by ACT. The same one-hot (untransposed
    Ot, also host-streamed) drives the scatter-sum matmuls.
  - u = xl[src]+xr[dst] on DVE; logits = per-head tree-reduction of
    att * leaky_relu(u) (ACT Prelu + DVE); softmax needs no max subtraction at
    these magnitudes. Since softmax weights sum to 1, we scatter exp(logit)*u
    and subtract xr once per destination at the end. exp runs on the tiny
    [P, tb*H, 2] logit pair (not the 32x broadcast).
  - Mean-pool via per-block one-hot graph-selector matmuls into a persistent
    PSUM accumulator, an 8-core AllReduce, and a tiny MLP.
"""

import math
import sys

sys.path.insert(0, "/opt/trn_rl_repo")

import ml_dtypes
import numpy as np

import concourse.bacc as bacc
import concourse.bass as bass
import concourse.mybir as mybir
import concourse.tile as tile
from concourse import library_config
from concourse.vector_clock import ScopedClock

BF16 = ml_dtypes.bfloat16

# ---------------------------------------------------------------------------
# Patches for walrus' one-sync-wait-per-instruction limit.
# ---------------------------------------------------------------------------


def _drain_and_barrier(self, tick_clock, wait_clock):
    nc = self.nc
    probe = nc.sync.nop(nofuse=True, hint="drain_wait_split")
    wait_clock.add_sem_waits(probe.ins, ScopedClock({None: tick_clock.global_clock}))
    si = probe.ins.sync_info
    waits = list(si.on_wait) if si and si.on_wait else []
    if len(waits) > 1:
        si.on_wait = waits[:1]
        for w in waits[1:]:
            extra = nc.sync.nop(nofuse=True, hint="drain_wait_split")
            extra.ins.sync_info = type(si)(on_wait=[w], on_update=[])
    nc.sync.drain()
    nc.all_engine_barrier()
    assert self.sems is not None
    popped = nc._tile_sem_poison_stack.pop()
    assert popped is self._sem_poison
    nc.clear_and_free_semaphores(list(self.sems.allocated().values()))
    nc.all_engine_barrier()


tile.TileContext._drain_and_barrier = _drain_and_barrier


def _split_waits(nc):
    """walrus codegen accepts one sync-wait command per instruction; Tile can
    emit several. Hoist extras onto preceding same-engine NoOps."""
    for bb in nc.main_func.blocks:
        out = []
        for ins in bb.instructions:
            si = ins.sync_info
            waits = list(si.on_wait) if si and si.on_wait else []
            if len(waits) > 1:
                for w in waits[:-1]:
                    nop = mybir.InstNoOp(
                        name=nc.get_next_instruction_name(), ins=[], outs=[]
                    )
                    nop.engine = ins.engine
                    nop.sync_info = mybir.SyncInfo(on_wait=[w], on_update=[])
                    nc.register_instruction(nop)
                    out.append(nop)
                si.on_wait = [waits[-1]]
            out.append(ins)
        bb.instructions = out


# ---------------------------------------------------------------------------
# Model dimensions (hardcoded per problem spec)
# ---------------------------------------------------------------------------
N = 50000
E = 800000
G = 128
H = 8
C = 32
HC = H * C  # 256
P = 128
NCORES = 8
SLOTS = 127  # real slots per block (slot 127 = garbage)
MAXI16 = 25600  # table-piece size for int16 gather indices
NPC = N // NCORES  # 6250
NBLK = (NPC + SLOTS - 1) // SLOTS  # 50
CAP = NBLK * P  # 6400
CAPEXT = NCORES * CAP  # 51200
CH = 5  # gather tiles per dma_gather call
NQ = 4  # SWDGE queues to rotate gathers over


class Cfg:
    def __init__(self, tba, tbb):
        self.tba = tba
        self.tbb = tbb
        self.tb = tba + tbb
        self.ttot = NBLK * self.tb


# ---------------------------------------------------------------------------
# Host-side preprocessing
# ---------------------------------------------------------------------------


def _f32(a):
    return np.ascontiguousarray(a, dtype=np.float32)


def _bf(a):
    return np.ascontiguousarray(np.asarray(a, dtype=np.float32).astype(BF16))


def _wrap_idx(ids):
    """Gather-index list -> [128, len/16] int16 in the SWDGE wrap layout
    (idx j read from [j % 16, j // 16], replicated over the 8 Q7 cores)."""
    ids = np.asarray(ids, np.int16)
    assert len(ids) % 16 == 0
    w = ids.reshape(-1, 16).T  # [16, s]
    return np.tile(w, (8, 1))  # [128, s]


def _plan_blocks(edge_index):
    """LPT-pack dst nodes into blocks; return assignment + per-core edge
    structures + uniform tile counts."""
    src = np.concatenate([edge_index[0], np.arange(N)]).astype(np.int64)
    dst = np.concatenate([edge_index[1], np.arange(N)]).astype(np.int64)
    pieceB = (src // NPC) >= (NCORES // 2)
    dega = np.bincount(dst[~pieceB], minlength=N)
    degb = np.bincount(dst[pieceB], minlength=N)

    blk_of = np.empty(N, np.int64)
    slot_of = np.empty(N, np.int64)
    for c in range(NCORES):
        lo = c * NPC
        da = dega[lo : lo + NPC].astype(np.float64)
        db = degb[lo : lo + NPC].astype(np.float64)
        order = np.argsort(-(da + db), kind="stable")
        blk_a = np.zeros(NBLK)
        blk_b = np.zeros(NBLK)
        blk_n = np.zeros(NBLK, np.int64)
        for i in order:
            cost = np.maximum(blk_a + da[i], blk_b + db[i])
            cost[blk_n >= SLOTS] = np.inf
            j = int(np.argmin(cost))
            blk_of[lo + i] = j
            slot_of[lo + i] = blk_n[j]
            blk_a[j] += da[i]
            blk_b[j] += db[i]
            blk_n[j] += 1
    row_of = blk_of * P + slot_of  # within-core table row
    ext_row = (np.arange(N) // NPC) * CAP + row_of  # global table row

    # per-(core, block, piece) edge lists
    esrow = ext_row[src]
    eslot = slot_of[dst]
    eblk = blk_of[dst]
    ecore = dst // NPC
    percore = []
    na = np.zeros((NCORES, NBLK), int)
    nb_ = np.zeros((NCORES, NBLK), int)
    for c in range(NCORES):
        blocks = []
        selc = ecore == c
        for b in range(NBLK):
            sel = selc & (eblk == b)
            sa = sel & ~pieceB
            sb = sel & pieceB
            ra, la = esrow[sa], eslot[sa]
            rb, lb = esrow[sb] - MAXI16, eslot[sb]
            blocks.append((ra, la, rb, lb))
            na[c, b] = len(ra)
            nb_[c, b] = len(rb)
        percore.append(blocks)
    tba = int(math.ceil(na.max() / P))
    tbb = int(math.ceil(nb_.max() / P))
    return percore, row_of, Cfg(tba, tbb)


def _prep(x, batch, u, weights, cfg: Cfg, percore, row_of):
    att1 = weights["att1"]
    att2 = weights["att2"]

    def att_rep(att):
        return _bf(np.broadcast_to(att.reshape(-1), (P, HC)))

    tba, tbb, tb = cfg.tba, cfg.tbb, cfg.tb
    iota = np.arange(P)

    maps = []
    for c in range(NCORES):
        m = {}
        lo = c * NPC
        ixa = np.zeros((P, NBLK * tba * 8), np.int16)
        ixb = np.zeros((P, NBLK * tbb * 8), np.int16)
        Ot = np.zeros((P, NBLK * tb * P), BF16)
        OtT = np.zeros((P, NBLK * tb * P), BF16)
        for b in range(NBLK):
            ra, la, rb, lb = percore[c][b]
            ia = np.zeros(tba * P, np.int64)
            ia[: len(ra)] = ra
            ib = np.zeros(tbb * P, np.int64)
            ib[: len(rb)] = rb
            ixa[:, b * tba * 8 : (b + 1) * tba * 8] = _wrap_idx(ia)
            ixb[:, b * tbb * 8 : (b + 1) * tbb * 8] = _wrap_idx(ib)
            slots = np.full(tb * P, 127, np.int64)
            slots[: len(la)] = la
            slots[tba * P : tba * P + len(lb)] = lb
            oh = (slots[:, None] == iota[None, :]).astype(BF16)  # [tb*P, P]
            oh = oh.reshape(tb, P, P)
            cols = slice(b * tb * P, (b + 1) * tb * P)
            Ot[:, cols] = oh.transpose(1, 0, 2).reshape(P, tb * P)
            OtT[:, cols] = oh.transpose(2, 0, 1).reshape(P, tb * P)
        m["ixa"] = ixa
        m["ixb"] = ixb
        m["Ot"] = np.ascontiguousarray(Ot)
        m["OtT"] = np.ascontiguousarray(OtT)

        rows = row_of[lo : lo + NPC]
        xs = np.zeros((CAP, x.shape[1]), np.float32)
        xs[rows] = x[lo : lo + NPC]
        m["xT"] = _bf(xs.T)

        gsel = np.zeros((CAP, G), np.float32)
        gsel[rows, np.asarray(batch[lo : lo + NPC])] = 1.0
        m["gsel"] = _bf(gsel)
        maps.append(m)

    counts = np.bincount(np.asarray(batch), minlength=G).astype(np.float32)
    shared = {
        "Wl1": _bf(weights["Wl1"]),
        "Wr1": _bf(weights["Wr1"]),
        "Wl2": _bf(weights["Wl2"]),
        "Wr2": _bf(weights["Wr2"]),
        "att1r": att_rep(att1),
        "att2r": att_rep(att2),
        "b1r": _bf(np.broadcast_to(weights["b1"], (P, HC))),
        "b2r": _bf(np.broadcast_to(weights["b2"], (P, HC))),
        "ident": _bf(np.eye(P, dtype=np.float32)),
        "crecip": _f32((1.0 / np.maximum(counts, 1.0)).reshape(G, 1)),
        "Wlin1": _bf(weights["W_lin1"]),
        "blin1r": _f32(np.broadcast_to(weights["b_lin1"], (G, 64))),
        "Wout": _bf(weights["W_out"]),
        "boutr": _f32(np.full((G, 1), float(weights["b_out"][0]), np.float32)),
        "ub": _bf(u),
    }
    for m in maps:
        m.update(shared)
    return maps


# ---------------------------------------------------------------------------
# Device program
# ---------------------------------------------------------------------------


def _bcast_mid(ap, reps):
    return ap.unsqueeze(1).broadcast_to([ap.shape[0], reps, ap.shape[1]])


def _build(cfg: Cfg, in_dim=3, use_bias=False):
    dt = mybir.dt
    bf = dt.bfloat16
    f32 = dt.float32
    nc = bacc.Bacc(None, num_swdge_queues=NQ) if NQ > 1 else bacc.Bacc(None)
    groups = [list(range(NCORES))]
    tba, tbb, tb = cfg.tba, cfg.tbb, cfg.tb

    def prm(name, shape, dtype):
        return nc.declare_dram_parameter(name, list(shape), dtype, isOutput=False)

    xT = prm("xT", [in_dim, CAP], bf)
    ixa = prm("ixa", [P, NBLK * tba * 8], dt.int16)
    ixb = prm("ixb", [P, NBLK * tbb * 8], dt.int16)
    Otp = prm("Ot", [P, NBLK * tb * P], bf)
    OtTp = prm("OtT", [P, NBLK * tb * P], bf)
    Wl1p = prm("Wl1", [in_dim, HC], bf)
    Wr1p = prm("Wr1", [in_dim, HC], bf)
    Wl2p = prm("Wl2", [HC, HC], bf)
    Wr2p = prm("Wr2", [HC, HC], bf)
    att1r = prm("att1r", [P, HC], bf)
    att2r = prm("att2r", [P, HC], bf)
    b1r = prm("b1r", [P, HC], bf)
    b2r = prm("b2r", [P, HC], bf)
    identp = prm("ident", [P, P], bf)
    gselp = prm("gsel", [CAP, G], bf)
    crecip = prm("crecip", [G, 1], f32)
    Wlin1 = prm("Wlin1", [HC, 64], bf)
    blin1r = prm("blin1r", [G, 64], f32)
    Woutp = prm("Wout", [64 + 3, 1], bf)
    boutr = prm("boutr", [G, 1], f32)
    ub = prm("ub", [G, 3], bf)
    out_g = nc.declare_dram_parameter("out_g", [G, 1], f32, isOutput=True)

    with tile.TileContext(nc) as tc:
        with (
            tc.tile_pool(name="const", bufs=1) as constp,
            tc.tile_pool(name="meta", bufs=3) as metap,
            tc.tile_pool(name="gbuf", bufs=2) as gbufp,
            tc.tile_pool(name="work", bufs=2) as workp,
            tc.tile_pool(name="small", bufs=3) as smallp,
            tc.tile_pool(name="psU", bufs=2, space="PSUM") as psU,
            tc.tile_pool(name="psA", bufs=1, space="PSUM") as psA,
            tc.tile_pool(name="psB", bufs=2, space="PSUM") as psB,
            tc.tile_pool(name="psG", bufs=1, space="PSUM") as psG,
            tc.tile_pool(name="dram", bufs=1, space="DRAM") as dram,
        ):
            # ---- constants to SBUF ----
            def cload(p):
                t = constp.tile([p.shape[0], p.shape[1]], p.dtype, name=p.name + "_s")
                nc.sync.dma_start(out=t[:], in_=p[:])
                return t

            def cload_k(p):
                nk = (p.shape[0] + P - 1) // P
                out = []
                for kt in range(nk):
                    rows = slice(kt * P, min((kt + 1) * P, p.shape[0]))
                    t = constp.tile(
                        [rows.stop - rows.start, p.shape[1]], p.dtype,
                        name=f"{p.name}_s{kt}",
                    )
                    nc.sync.dma_start(out=t[:], in_=p[rows, :])
                    out.append(t)
                return out

            xT_s = cload(xT)
            Wl1_s = cload_k(Wl1p)
            Wr1_s = cload_k(Wr1p)
            Wl2_s = cload_k(Wl2p)
            Wr2_s = cload_k(Wr2p)
            att1r_s = cload(att1r)
            att2r_s = cload(att2r)
            b1r_s = cload(b1r)
            b2r_s = cload(b2r)
            ident_s = cload(identp)
            crecip_s = cload(crecip)
            Wlin1_s = cload_k(Wlin1)
            blin1r_s = cload(blin1r)
            Wout_s = cload(Woutp)
            boutr_s = cload(boutr)
            ub_s = cload(ub)

            # ---- internal DRAM ----
            xl1_own = dram.tile([CAP, HC], bf)
            xr1_tab = dram.tile([CAP, HC], bf)
            xl1_ext = dram.tile([CAPEXT, HC], bf, addr_space="Shared")
            xl2_own = dram.tile([CAP, HC], bf)
            xr2_tab = dram.tile([CAP, HC], bf)
            xl2_ext = dram.tile([CAPEXT, HC], bf, addr_space="Shared")
            gp_in = dram.tile([G, HC], f32)
            gp_out = dram.tile([G, HC], f32, addr_space="Shared")

            A_ = mybir.AluOpType
            AF = mybir.ActivationFunctionType

            # ================= node tables =================
            def node_tables(lhsT_tiles, W_s, tab):
                for b in range(NBLK):
                    ps = psA.tile([P, HC], f32, tag="a")
                    lts = lhsT_tiles(b)
                    assert len(lts) == len(W_s)
                    for i, lt in enumerate(lts):
                        nc.tensor.matmul(
                            ps[:], lhsT=lt, rhs=W_s[i][:],
                            start=(i == 0), stop=(i == len(lts) - 1),
                        )
                    ev = smallp.tile([P, HC], bf, tag="tabev")
                    nc.scalar.activation(out=ev[:], in_=ps[:], func=AF.Copy)
                    nc.sync.dma_start(out=tab[b * P : (b + 1) * P, :], in_=ev[:])

            def x_lhsT(b):
                return [xT_s[:, b * P : (b + 1) * P]]

            # ================= edge pipeline =================
            qctr = [0]

            def edge_layer(xl_ext, xr_tab, attr_s, br_s, layer):
                gpool_ps = None
                if layer == 2:
                    gpool_ps = psG.tile([G, HC], f32, name=f"gpool_ps{layer}")

                def front(b):
                    """Block prologue: meta loads, gathers, one-hot xr expand.
                    Emitted one block ahead so these fill other blocks' stalls."""
                    st = {}
                    cols = slice(b * tb * P, (b + 1) * tb * P)
                    Ot_blk = metap.tile([P, tb * P], bf, tag="Ot")
                    nc.sync.dma_start(out=Ot_blk[:], in_=Otp[:, cols])
                    OtT_blk = metap.tile([P, tb * P], bf, tag="OtT")
                    nc.sync.dma_start(out=OtT_blk[:], in_=OtTp[:, cols])
                    ixa_t = metap.tile([P, tba * 8], dt.int16, tag="ixa")
                    nc.sync.dma_start(
                        out=ixa_t[:], in_=ixa[:, b * tba * 8 : (b + 1) * tba * 8]
                    )
                    ixb_t = metap.tile([P, tbb * 8], dt.int16, tag="ixb")
                    nc.sync.dma_start(
                        out=ixb_t[:], in_=ixb[:, b * tbb * 8 : (b + 1) * tbb * 8]
                    )
                    xr_blk = metap.tile([P, HC], bf, tag="xr")
                    nc.sync.dma_start(out=xr_blk[:], in_=xr_tab[b * P : (b + 1) * P, :])
                    if layer == 2:
                        gsel_blk = metap.tile([P, G], bf, tag="gselb")
                        nc.sync.dma_start(
                            out=gsel_blk[:], in_=gselp[b * P : (b + 1) * P, :]
                        )
                        st["gsel"] = gsel_blk

                    gxl = gbufp.tile([P, tb, HC], bf, tag="gxl")

                    def chunked_gather(dst_t0, n_tiles, table, idxt):
                        for q0 in range(0, n_tiles, CH):
                            q1 = min(q0 + CH, n_tiles)
                            nc.gpsimd.dma_gather(
                                out_ap=gxl[:, dst_t0 + q0 : dst_t0 + q1, :],
                                in_ap=table,
                                idxs_ap=idxt[:, q0 * 8 : q1 * 8],
                                num_idxs=(q1 - q0) * P, num_idxs_reg=(q1 - q0) * P,
                                elem_size=HC,
                                queue_num=qctr[0] % NQ,
                            )
                            qctr[0] += 1

                    chunked_gather(0, tba, xl_ext[0:MAXI16, :], ixa_t)
                    chunked_gather(tba, tbb, xl_ext[MAXI16:CAPEXT, :], ixb_t)

                    # one-hot xr expand; then u = xl+xr and leaky_relu per
                    # group so the whole logit front-end pipelines group-wise.
                    xre = gbufp.tile([P, tb, HC], bf, tag="xre")
                    grps = []
                    for t0 in range(0, tb, 4):
                        k = min(4, tb - t0)
                        grps.append((t0, k))
                        ps = psU.tile([P, 4, HC], f32, tag="u")
                        for u_ in range(k):
                            t_ = t0 + u_
                            nc.tensor.matmul(
                                ps[:, u_, :],
                                lhsT=OtT_blk[:, t_ * P : (t_ + 1) * P],
                                rhs=xr_blk[:], start=True, stop=True,
                            )
                        if t0 >= 8:
                            nc.vector.tensor_copy(
                                out=xre[:, t0 : t0 + k, :], in_=ps[:, 0:k, :]
                            )
                        else:
                            nc.scalar.activation(
                                out=xre[:, t0 : t0 + k, :], in_=ps[:, 0:k, :],
                                func=AF.Copy,
                            )
                    st.update(Ot=Ot_blk, gxl=gxl, xre=xre)
                    return st

                def back(b, st):
                    Ot_blk, gxl, xre = st["Ot"], st["gxl"], st["xre"]
                    ut = workp.tile([P, tb, HC], bf, tag="ut")
                    nc.vector.tensor_tensor(out=ut[:], in0=gxl[:], in1=xre[:], op=A_.add)
                    ft = workp.tile([P, tb, HC], bf, tag="ft")
                    Pt = workp.tile([P, tb, HC], bf, tag="Pt")
                    th = tb // 2
                    for lo, hi in ((0, th), (th, tb)):
                        nc.scalar.activation(
                            out=ft[:, lo:hi, :], in_=ut[:, lo:hi, :],
                            func=AF.Prelu, alpha=0.2,
                        )
                        nc.vector.tensor_tensor(
                            out=Pt[:, lo:hi, :], in0=ft[:, lo:hi, :],
                            in1=_bcast_mid(attr_s[:], hi - lo), op=A_.mult,
                        )
                    v = Pt[:].rearrange("p t (h c) -> p (t h) c", h=H)
                    t1 = workp.tile([P, tb * H, 16], bf, tag="t1")
                    nc.vector.tensor_tensor(out=t1[:], in0=v[:, :, 0:16], in1=v[:, :, 16:32], op=A_.add)
                    t2 = workp.tile([P, tb * H, 8], bf, tag="t2")
                    nc.vector.tensor_tensor(out=t2[:], in0=t1[:, :, 0:8], in1=t1[:, :, 8:16], op=A_.add)
                    t3 = workp.tile([P, tb * H, 4], bf, tag="t3")
                    nc.vector.tensor_tensor(out=t3[:], in0=t2[:, :, 0:4], in1=t2[:, :, 4:8], op=A_.add)
                    t4 = workp.tile([P, tb * H, 2], bf, tag="t4")
                    nc.vector.tensor_tensor(out=t4[:], in0=t3[:, :, 0:2], in1=t3[:, :, 2:4], op=A_.add)
                    lg2 = workp.tile([P, tb * H, 2], bf, tag="lg2")
                    nc.vector.tensor_tensor(
                        out=lg2[:],
                        in0=t4[:, :, 0:1].broadcast_to([P, tb * H, 2]),
                        in1=t4[:, :, 1:2].broadcast_to([P, tb * H, 2]),
                        op=A_.add,
                    )
                    ex2 = workp.tile([P, tb * H, 2], bf, tag="ex2")
                    nc.scalar.activation(out=ex2[:], in_=lg2[:], func=AF.Exp)
                    exv = ex2[:].rearrange("p (t h) j -> p t h j", t=tb)
                    msg = workp.tile([P, tb, HC], bf, tag="msg")
                    nc.vector.tensor_tensor(
                        out=msg[:].rearrange("p t (h k j) -> p (t h) k j", h=H, j=2),
                        in0=gxl[:].rearrange("p t (h k j) -> p (t h) k j", h=H, j=2),
                        in1=ex2[:].unsqueeze(2).broadcast_to([P, tb * H, 16, 2]),
                        op=A_.mult,
                    )

                    psacc = psB.tile([P, HC + H], f32, tag="b")
                    acc = psacc[:, 0:HC]
                    accd = psacc[:, HC : HC + H]
                    for t in range(tb):
                        Ot_t = Ot_blk[:, t * P : (t + 1) * P]
                        nc.tensor.matmul(
                            acc, lhsT=Ot_t, rhs=msg[:, t, :],
                            start=(t == 0), stop=(t == tb - 1),
                        )
                        # start=False even at t==0: acc's start=True already
                        # cleared the whole bank's has_written bits, so the
                        # first accd matmul overwrites (bit unset) rather than
                        # accumulating onto garbage; a second start=True here
                        # would re-clear the bank and drop acc's tile-0 sums.
                        nc.tensor.matmul(
                            accd, lhsT=Ot_t, rhs=exv[:, t, :, 0],
                            start=False, stop=(t == tb - 1),
                        )

                    denom = smallp.tile([P, H], f32, tag="denom")
                    nc.vector.tensor_scalar(
                        out=denom[:], in0=accd, scalar1=1e-20, scalar2=None,
                        op0=A_.max,
                    )
                    rec = smallp.tile([P, H], f32, tag="rec")
                    nc.vector.reciprocal(out=rec[:], in_=denom[:])
                    hsc = smallp.tile([P, HC], bf, tag="hsc")
                    nc.vector.tensor_tensor(
                        out=hsc[:].rearrange("p (h c) -> p h c", h=H),
                        in0=acc.rearrange("p (h c) -> p h c", h=H),
                        in1=rec[:].to_broadcast([P, H, C]),
                        op=A_.mult,
                    )
                    if use_bias:
                        hfin = smallp.tile([P, HC], bf, tag="hfin")
                        nc.vector.tensor_tensor(
                            out=hfin[:], in0=hsc[:], in1=br_s[:], op=A_.add
                        )
                        hsc = hfin
                    hout = smallp.tile([P, HC], bf, tag="hout")
                    nc.vector.tensor_scalar(
                        out=hout[:], in0=hsc[:], scalar1=0.0, scalar2=None,
                        op0=A_.max,
                    )

                    if layer == 1:
                        # transpose h1 block and compute layer-2 node tables
                        # inline (h1T never round-trips through DRAM).
                        tps = []
                        for kt in range(2):
                            tp = psA.tile([P, P], bf, tag="a")
                            nc.tensor.transpose(
                                out=tp[:], in_=hout[:, kt * P : (kt + 1) * P],
                                identity=ident_s[:],
                            )
                            t_ = smallp.tile([P, P], bf, tag="htps")
                            nc.scalar.activation(out=t_[:], in_=tp[:], func=AF.Copy)
                            tps.append(t_)
                        for W_s, tab in ((Wl2_s, xl2_own), (Wr2_s, xr2_tab)):
                            ps2 = psA.tile([P, HC], f32, tag="a")
                            for i in range(2):
                                nc.tensor.matmul(
                                    ps2[:], lhsT=tps[i][:], rhs=W_s[i][:],
                                    start=(i == 0), stop=(i == 1),
                                )
                            ev = smallp.tile([P, HC], bf, tag="tabev")
                            nc.scalar.activation(out=ev[:], in_=ps2[:], func=AF.Copy)
                            nc.sync.dma_start(
                                out=tab[b * P : (b + 1) * P, :], in_=ev[:]
                            )
                    else:
                        nc.tensor.matmul(
                            gpool_ps[:], lhsT=st["gsel"][:], rhs=hout[:],
                            start=(b == 0), stop=(b == NBLK - 1),
                        )

                st = front(0)
                for b in range(NBLK):
                    nxt = front(b + 1) if b + 1 < NBLK else None
                    back(b, st)
                    st = nxt
                return gpool_ps

            # ================= layer 1 =================
            node_tables(x_lhsT, Wl1_s, xl1_own)
            nc.gpsimd.collective_compute(
                "AllGather", A_.bypass, replica_groups=groups,
                ins=[xl1_own.opt()], outs=[xl1_ext.opt()],
            )
            node_tables(x_lhsT, Wr1_s, xr1_tab)
            edge_layer(xl1_ext, xr1_tab, att1r_s, b1r_s, layer=1)

            # ================= layer 2 =================
            nc.gpsimd.collective_compute(
                "AllGather", A_.bypass, replica_groups=groups,
                ins=[xl2_own.opt()], outs=[xl2_ext.opt()],
            )
            gpool_ps = edge_layer(xl2_ext, xr2_tab, att2r_s, b2r_s, layer=2)

            # ================= pool + MLP =================
            gsum = smallp.tile([G, HC], f32, tag="gsum")
            nc.scalar.activation(out=gsum[:], in_=gpool_ps[:], func=AF.Copy)
            nc.sync.dma_start(out=gp_in[:], in_=gsum[:])
            nc.gpsimd.collective_compute(
                "AllReduce", A_.add, replica_groups=groups,
                ins=[gp_in.opt()], outs=[gp_out.opt()],
            )
            gsum2 = smallp.tile([G, HC], f32, tag="gsum2")
            nc.sync.dma_start(out=gsum2[:], in_=gp_out[:])
            gmean = smallp.tile([G, HC], bf, tag="gmean")
            nc.vector.tensor_scalar(
                out=gmean[:], in0=gsum2[:], scalar1=crecip_s[:, 0:1], scalar2=None,
                op0=A_.mult,
            )
            gT = []
            for kt in range(2):
                tp = psA.tile([P, G], bf, tag="a")
                nc.tensor.transpose(
                    out=tp[:], in_=gmean[:, kt * P : (kt + 1) * P], identity=ident_s[:]
                )
                gkt = smallp.tile([P, G], bf, tag="gT", name=f"gT{kt}")
                nc.scalar.activation(out=gkt[:], in_=tp[:], func=AF.Copy)
                gT.append(gkt)
            lin_ps = psB.tile([G, 64], f32, tag="b")
            for kt in range(2):
                nc.tensor.matmul(
                    lin_ps[:], lhsT=gT[kt][:], rhs=Wlin1_s[kt][:],
                    start=(kt == 0), stop=(kt == 1),
                )
            lin = smallp.tile([G, 64], f32, tag="lin")
            nc.vector.tensor_tensor(out=lin[:], in0=lin_ps[:], in1=blin1r_s[:], op=A_.add)
            glu = smallp.tile([G, P], bf, tag="glu")
            nc.scalar.activation(out=glu[:, 0:64], in_=lin[:], func=AF.Relu)
            nc.vector.tensor_copy(out=glu[:, 64:67], in_=ub_s[:])
            nc.gpsimd.memset(glu[:, 67:P], 0.0)
            tp = psA.tile([P, G], bf, tag="a")
            nc.tensor.transpose(out=tp[:], in_=glu[:], identity=ident_s[:])
            gluT = smallp.tile([P, G], bf, tag="gluT")
            nc.scalar.activation(out=gluT[:], in_=tp[:], func=AF.Copy)
            out_ps = psB.tile([G, 1], f32, tag="b")
            nc.tensor.matmul(
                out_ps[:], lhsT=gluT[0:67, :], rhs=Wout_s[:], start=True, stop=True
            )
            outs = smallp.tile([G, 1], f32, tag="outs")
            nc.vector.tensor_tensor(out=outs[:], in0=out_ps[:], in1=boutr_s[:], op=A_.add)
            nc.sync.dma_start(out=out_g[:], in_=outs[:])

    nc.compile()
    _split_waits(nc)
    return nc


# ---------------------------------------------------------------------------
# Entry point
# ---------------------------------------------------------------------------


def kernel(**inputs):
    import os

    from concourse.bass_utils import run_bass_kernel_spmd

    x = np.asarray(inputs["x"], np.float32)
    edge_index = np.asarray(inputs["edge_index"], np.int64)
    batch = np.asarray(inputs["batch"], np.int64)
    u = np.asarray(inputs["u"], np.float32)
    weights = {
        k: np.asarray(inputs[k], np.float32)
        for k in ("Wl1", "Wr1", "att1", "b1", "Wl2", "Wr2", "att2", "b2",
                  "W_lin1", "b_lin1", "W_out", "b_out")
    }
    percore, row_of, cfg = _plan_blocks(edge_index)
    maps = _prep(x, batch, u, weights, cfg, percore, row_of)
    use_bias = bool(np.any(weights["b1"]) or np.any(weights["b2"]))
    nc = _build(cfg, in_dim=x.shape[1], use_bias=use_bias)
    trace = bool(os.environ.get("KERNEL_TRACE"))
    try:
        res = run_bass_kernel_spmd(nc, maps, list(range(NCORES)), trace=trace)
    except ModuleNotFoundError:
        res = run_bass_kernel_spmd(nc, maps, list(range(NCORES)))
    if trace and getattr(res, "exec_time_ns", None) is not None:
        print(f"HW exec time: {res.exec_time_ns} ns")
    return res.results[0]["out_g"].reshape(G).astype(np.float32)
